# revision 32
# baseline (speedup 1.0000x reference)
"""CQT (constant-Q transform) + amplitude_to_db kernel for Trainium2.

Full-input contract: kernel(x) takes x [32, 64000] f32 and returns
[32, 84, 126] f32, matching:

    frames = pad(x, n_fft//2)[:, t*HOP + n]          # [B, 126, 16384]
    cr/ci  = frames @ Kr.T / Ki.T                    # [B, 84, 126]
    mag    = sqrt(cr^2 + ci^2)
    out    = amplitude_to_db(mag, ref=max per item, amin=1e-5, top_db=80)

Sharding: pure data parallelism - 4 batch items per NeuronCore on 8 cores.

Per-core compute layout (mixed fp16 / fp8-DoubleRow):
  * One big matmul with K = n_fft = 16384 contracted in 128-row chunks;
    padded x stored column-major in SBUF so chunk c of frames^T is a strided
    AP view (HOP = 4*128).  All 4 items share each matmul via N = 504.
  * CQT kernel energy is extremely concentrated: the central 16 K-chunks
    hold 99.88% of the group-A (bins 0..63) weight energy.  Those 16 chunks
    plus all 5 group-B chunks (bins 64..83) run in fp16.  The remaining 76
    low-energy tail chunks run as 38 fp8e4m3 DoubleRow matmuls (two 128-row
    k-tiles per instruction = 2x PE throughput), with per-bin power-of-2
    weight scales to center the fp8 dynamic range.  All tail chunks have
    support only in bins 0..31, so the DR stationary is M=64; psA planes
    are [re-lo, im-lo, re-hi, im-hi] x 32 partitions to make that work.
    Measured dB-domain rel-L2 error of this split is ~4.4e-3 (gate 2e-2).
  * The A chain closes before the 5 B matmuls run, so the (critical) A
    epilogue overlaps them.  DMA pieces are blob-merged in consumption
    order; completion tracks issue order at ~250 GB/s aggregate.
  * dB epilogue: Square psA planes (4 ACT passes remapping to re^2/im^2 by
    bin), add, fused (descale, clamp-at-amin^2) 2-op, ACT Ln, per-item max
    via free-dim reduce + GpSimd partition all-reduce, then
    out = (ln(m2c) - ln(ref2c)) * 10/ln(10).
"""

import os
import numpy as np
import ml_dtypes

import concourse.bass as bass
import concourse.mybir as mybir
from concourse import bacc
from concourse import bass_isa
from concourse.bass_utils import run_bass_kernel_spmd

# ---- problem constants (hardcoded; must match the reference) ----
SR = 22050
HOP = 512
N_BINS = 84
BPO = 12
FMIN = 32.70319566257483
AMIN = 1e-5
TOP_DB = 80.0
B = 32
N_SAMP = 64000
N_CORES = 8
NI = B // N_CORES            # items per core = 4
T = 1 + N_SAMP // HOP        # 126 frames
DB_SCALE = 10.0 / np.log(10.0)
P = 128

SPLIT_BIN = 64               # group A: bins [0,64), group B: bins [64,84)
NB_BINS = N_BINS - SPLIT_BIN  # 20
H = SPLIT_BIN // 2           # 32

# if "1", the block does not wait for the output DMA completion semaphores;
# the framework postamble (all-engine barrier + sem-reset storm, ~7us) then
# overlaps the in-flight output DMA instead of serializing after it.
NO_OUT_WAIT = os.environ.get("CQT_NO_OUT_WAIT", "1") == "1"


def _build_cqt_kernels():
    """Same construction as the reference (nnAudio-style direct CQT bank)."""
    Q = 1.0 / (2.0 ** (1.0 / BPO) - 1.0)
    freqs = FMIN * 2.0 ** (np.arange(N_BINS) / BPO)
    lengths = np.ceil(Q * SR / freqs).astype(int)
    n_fft = int(2 ** np.ceil(np.log2(lengths.max())))
    K = np.zeros((N_BINS, n_fft), dtype=np.complex128)
    for k in range(N_BINS):
        L = int(lengths[k])
        t = np.arange(L) - (L - 1) / 2.0
        kern = np.hanning(L) * np.exp(2j * np.pi * freqs[k] * t / SR)
        kern /= np.abs(kern).sum()
        kern /= np.sqrt(L)
        s = (n_fft - L) // 2
        K[k, s:s + L] = kern
    return K.real.astype(np.float32), K.imag.astype(np.float32), n_fft


Kr, Ki, N_FFT = _build_cqt_kernels()
PAD = N_FFT // 2
FW = (N_SAMP + 2 * PAD) // P      # 628 free-dim width of column-major xp
QW = FW // 4                      # 157
NT = NI * T                       # 504
assert (N_SAMP + 2 * PAD) % P == 0 and HOP == 4 * P

# per-bin power-of-2 scale so scaled |w| peaks near 112 (fp8e4m3 max 240)
_WMAX = np.maximum(np.abs(Kr).max(1), np.abs(Ki).max(1))
BIN_SCALE = 2.0 ** np.floor(np.log2(224.0 / _WMAX / 2.0))

# ---- chunk geometry ----
# group A support: chunks [19, 109); central fp16 window [56, 72)
F0, F1 = 56, 72
CH16 = [56, 60, 64, 68, 57, 61, 65, 69, 58, 62, 66, 70, 59, 63, 67, 71]
assert sorted(CH16) == list(range(F0, F1))
CHB = [62, 63, 64, 65, 66]   # group B support chunks (run at the very end)
# fp8 DR pairs (c, c+1), c even: left tail [18,56), right tail [72,110);
# chunks 18 and 109 are zero-padded (outside the true support [19,109)).
_LEFT = [(c, c + 1) for c in range(18, 56, 2)]
_RIGHT = [(c, c + 1) for c in range(72, 110, 2)]
_ALLP = _LEFT + _RIGHT
PAIRS01 = [p for p in _ALLP if p[0] % 4 == 0]   # phases (0,1)
PAIRS23 = [p for p in _ALLP if p[0] % 4 == 2]   # phases (2,3)
PAIRS = PAIRS01 + PAIRS23                        # weight-pack order
NPAIR = len(PAIRS)
N01 = len(PAIRS01)
assert NPAIR == 19 + 19

# x16 is only read by the fp16 chunks (q0 in [14,17], so q in [14,143))
Q16LO, Q16HI = 14, 143
QW16 = Q16HI - Q16LO              # 129

# ---- blob layouts (f16 cols / fp8 cols), in consumption order ----
# b16: [ x16 phase0 (NI*QW16) | w16 16 chunks (16*128) | wb (5*64) ]
B16_X0 = 0
B16_W16 = NI * QW16
B16_WB = B16_W16 + 16 * P
B16_END = B16_WB + len(CHB) * 64
B16A_END = B16_W16 + 4 * P        # piece 1: x16p0 + w16 phase-0 chunks
# b8: [ x8 ph01 (2*NI*QW) | w8 pairs01 (19*128) | x8 ph23 | w8 pairs23 ]
B8_X01 = 0
B8_W01 = 2 * NI * QW
B8_X23 = B8_W01 + N01 * 2 * P
B8_W23 = B8_X23 + 2 * NI * QW
B8_END = B8_W23 + (NPAIR - N01) * 2 * P
W8S1 = 6                           # first fp8 slab: 6 pairs

f16 = mybir.dt.float16
bf16 = mybir.dt.bfloat16
fp8 = mybir.dt.float8e4
f32 = mybir.dt.float32
DR = mybir.MatmulPerfMode.DoubleRow
NP16 = np.float16
NPBF = ml_dtypes.bfloat16
NP8 = ml_dtypes.float8_e4m3


def _pack_weights():
    """psA plane layout: partitions [0:64) re bins 0..63, [64:128) im."""
    KrT = (Kr * BIN_SCALE[:, None]).T   # [N_FFT, 84] scaled
    KiT = (Ki * BIN_SCALE[:, None]).T

    w16 = np.zeros((P, 16 * P), np.float32)
    for j, c in enumerate(CH16):
        sl = slice(c * P, (c + 1) * P)
        w16[:, j * P: j * P + SPLIT_BIN] = KrT[sl, :SPLIT_BIN]
        w16[:, j * P + SPLIT_BIN:(j + 1) * P] = KiT[sl, :SPLIT_BIN]

    wb = np.zeros((P, len(CHB) * 64), np.float32)
    for j, c in enumerate(CHB):
        wb[:, j * 64: j * 64 + NB_BINS] = KrT[c * P:(c + 1) * P, SPLIT_BIN:]
        wb[:, j * 64 + 32: j * 64 + 32 + NB_BINS] = KiT[c * P:(c + 1) * P, SPLIT_BIN:]

    w8 = np.zeros((P, NPAIR * 2 * P), np.float32)
    for j, (ca, cb) in enumerate(PAIRS):
        for ti, c in ((0, ca), (1, cb)):
            if c < 19 or c > 108:
                continue   # zero-padded phantom chunk
            base = j * 2 * P + ti * P
            w8[:, base: base + SPLIT_BIN] = KrT[c * P:(c + 1) * P, :SPLIT_BIN]
            w8[:, base + SPLIT_BIN: base + P] = KiT[c * P:(c + 1) * P, :SPLIT_BIN]
    return w16.astype(NP16), wb.astype(NP16), w8.astype(NP8)


W16, WB, W8 = _pack_weights()

# per-partition descale vectors:
# col 1: B layout 1/c_k (rows 0:20 re bins 64..83, 32:52 im), ACT Square scale
# col 2: A layout 1/c_k^2 (rows 0:64, bin k=p), DVE post-add descale
CN = np.ones((P, 3), np.float32)
CN[:NB_BINS, 1] = 1.0 / BIN_SCALE[SPLIT_BIN:]
CN[32:32 + NB_BINS, 1] = 1.0 / BIN_SCALE[SPLIT_BIN:]
CN[:SPLIT_BIN, 2] = 1.0 / BIN_SCALE[:SPLIT_BIN] ** 2


def build_program():
    nc = bacc.Bacc("TRN2", target_bir_lowering=False, debug=False,
                   enable_asserts=True)

    b16_in = nc.dram_tensor("b16_in", [P, B16_END], f16,
                            kind="ExternalInput").ap()
    x16r_in = nc.dram_tensor("x16r_in", [3, P, NI * QW16], f16,
                             kind="ExternalInput").ap()
    b8_in = nc.dram_tensor("b8_in", [P, B8_END], fp8,
                           kind="ExternalInput").ap()
    cn_in = nc.dram_tensor("cn_in", [P, 3], f32, kind="ExternalInput").ap()
    out = nc.dram_tensor("out", [N_BINS, NT], f32, kind="ExternalOutput").ap()

    sb16 = nc.alloc_sbuf_tensor("sb16", [P, B16_END], f16).ap()
    xt16r = nc.alloc_sbuf_tensor("xt16r", [P, 3 * NI * QW16], f16).ap()
    sb8 = nc.alloc_sbuf_tensor("sb8", [P, B8_END], fp8).ap()
    cn = nc.alloc_sbuf_tensor("cn", [P, 3], f32).ap()
    junk = nc.alloc_sbuf_tensor("junk", [P, 512], f16).ap()
    sq0 = nc.alloc_sbuf_tensor("sq0", [SPLIT_BIN, NT], f32).ap()
    sq1 = nc.alloc_sbuf_tensor("sq1", [SPLIT_BIN, NT], f32).ap()
    tmpB = nc.alloc_sbuf_tensor("tmpB", [N_BINS, NT], f32).ap()
    m2 = nc.alloc_sbuf_tensor("m2", [N_BINS, NT], f32).ap()
    lnm = nc.alloc_sbuf_tensor("lnm", [N_BINS, NT], f32).ap()
    r1 = nc.alloc_sbuf_tensor("r1", [N_BINS, NI], f32).ap()
    rall = nc.alloc_sbuf_tensor("rall", [N_BINS, NI], f32).ap()
    lnr = nc.alloc_sbuf_tensor("lnr", [N_BINS, NI], f32).ap()
    db = nc.alloc_sbuf_tensor("db", [N_BINS, NT], f32).ap()
    lnwarm = nc.alloc_sbuf_tensor("lnwarm", [1, 2], f32).ap()

    psW = nc.alloc_psum_tensor("psW", [P, NT], f32).ap()
    psA = nc.alloc_psum_tensor("psA", [P, NT], f32).ap()
    psB = nc.alloc_psum_tensor("psB", [64, NT], f32).ap()

    s_b16a = nc.alloc_semaphore("s_b16a")
    s_b16b = nc.alloc_semaphore("s_b16b")
    s_x8a = nc.alloc_semaphore("s_x8a")
    s_w8a = nc.alloc_semaphore("s_w8a")
    s_w8b = nc.alloc_semaphore("s_w8b")
    s_b8b = nc.alloc_semaphore("s_b8b")
    s16p = [nc.alloc_semaphore(f"s16p{r}") for r in (1, 2, 3)]
    s_ic = nc.alloc_semaphore("s_ic")
    s_mi = nc.alloc_semaphore("s_mi")
    s_pe = nc.alloc_semaphore("s_pe")     # 1 = psA final, 2 = psB final
    s_a = nc.alloc_semaphore("s_a")       # 1 = A squares done, 2 = B squares
    s_vB = nc.alloc_semaphore("s_vB")     # m2c B ready
    s_vA = nc.alloc_semaphore("s_vA")     # m2c A ready
    s_r = nc.alloc_semaphore("s_r")       # r1 halves done
    s_g = nc.alloc_semaphore("s_g")       # allreduce done
    s_lnr = nc.alloc_semaphore("s_lnr")   # lnr (and lnm) ready
    s_db = nc.alloc_semaphore("s_db")     # db ready for output
    s_out = nc.alloc_semaphore("s_out")
    s_out2 = nc.alloc_semaphore("s_out2")

    xv16p0 = sb16[:, B16_X0:B16_X0 + NI * QW16].rearrange(
        "p (i q) -> p i q", i=NI)
    w16t = sb16[:, B16_W16:B16_W16 + 16 * P]
    wbt = sb16[:, B16_WB:B16_WB + len(CHB) * 64]
    xv16r = xt16r.rearrange("p (r i q) -> p r i q", r=3, i=NI)
    xv8a = sb8[:, B8_X01:B8_X01 + 2 * NI * QW].rearrange(
        "p (j i q) -> p j i q", j=2, i=NI)
    xv8b = sb8[:, B8_X23:B8_X23 + 2 * NI * QW].rearrange(
        "p (j i q) -> p j i q", j=2, i=NI)
    wv8a = sb8[:, B8_W01:B8_W01 + N01 * 2 * P].rearrange(
        "p (j two m) -> p j two m", two=2, m=P)
    wv8b = sb8[:, B8_W23:B8_W23 + (NPAIR - N01) * 2 * P].rearrange(
        "p (j two m) -> p j two m", two=2, m=P)
    psAv = psA.rearrange("p (i t) -> p i t", i=NI)
    psBv = psB.rearrange("p (i t) -> p i t", i=NI)

    def rhs16(c):
        r, q0 = c % 4, c // 4 - Q16LO
        if r == 0:
            return xv16p0[:, :, q0: q0 + T]
        return xv16r[:, r - 1, :, q0: q0 + T]

    def rhs8(pair):
        c = pair[0]
        rp, q0 = c // 2 % 2, c // 4
        xv = xv8a if rp == 0 else xv8b
        return xv[:, :, :, q0: q0 + T]

    Ln = mybir.ActivationFunctionType.Ln
    Square = mybir.ActivationFunctionType.Square
    AMIN2 = float(AMIN) ** 2

    with nc.Block(no_gpsimd_drain=True) as block:

        @block.sync
        def _(sync):
            sync.dma_start(sb16[:, :B16A_END], b16_in[:, :B16A_END]
                           ).then_inc(s_b16a, 16)
            sync.dma_start(
                sb8[:, B8_W01 + W8S1 * 256:B8_X23],
                b8_in[:, B8_W01 + W8S1 * 256:B8_X23]).then_inc(s_w8b, 16)
            sync.dma_start(xt16r[:, NI * QW16:2 * NI * QW16], x16r_in[1]
                           ).then_inc(s16p[1], 16)
            sync.dma_start(cn[:], cn_in).then_inc(s_ic, 16)
            sync.wait_ge(s_db, 1)
            sync.dma_start(out[0:42], db[0:42]).then_inc(s_out, 16)
            if not NO_OUT_WAIT:
                sync.wait_ge(s_out, 16)

        @block.scalar
        def _(scalar):
            scalar.dma_start(sb8[:, B8_X01:B8_W01], b8_in[:, B8_X01:B8_W01]
                             ).then_inc(s_x8a, 16)
            scalar.dma_start(sb16[:, B16A_END:], b16_in[:, B16A_END:]
                             ).then_inc(s_b16b, 16)
            scalar.dma_start(xt16r[:, 2 * NI * QW16:], x16r_in[2]
                             ).then_inc(s16p[2], 16)
            # preload BOTH act table slots (Ln set + Square set)
            scalar.activation(lnwarm[:, 0:1], nc.const_aps.tensor(1.0, (1, 1)), Ln)
            scalar.activation(lnwarm[:, 1:2], nc.const_aps.tensor(1.0, (1, 1)),
                              Square)
            # A epilogue: sq0 = re^2, sq1 = im^2
            scalar.wait_ge(s_pe, 1)
            scalar.activation(sq0[:], psA[0:SPLIT_BIN], Square)
            scalar.activation(sq1[:], psA[SPLIT_BIN:], Square).then_inc(s_a)
            # B epilogue
            scalar.wait_ge(s_pe, 2)
            scalar.wait_ge(s_ic, 16)
            scalar.activation(m2[SPLIT_BIN:], psB[0:NB_BINS], Square,
                              scale=cn[0:NB_BINS, 1:2])
            scalar.activation(tmpB[SPLIT_BIN:], psB[32:32 + NB_BINS], Square,
                              scale=cn[32:32 + NB_BINS, 1:2]).then_inc(s_a)
            scalar.wait_ge(s_vA, 1)
            scalar.activation(lnm[:SPLIT_BIN], m2[:SPLIT_BIN], Ln)
            scalar.wait_ge(s_vB, 1)
            scalar.activation(lnm[SPLIT_BIN:], m2[SPLIT_BIN:], Ln)
            scalar.wait_ge(s_g, 1)
            scalar.activation(lnr[:], rall[:], Ln).then_inc(s_lnr)

        @block.gpsimd
        def _(gpsimd):
            gpsimd.dma_start(sb8[:, B8_W01:B8_W01 + W8S1 * 256],
                             b8_in[:, B8_W01:B8_W01 + W8S1 * 256]
                             ).then_inc(s_w8a, 16)
            gpsimd.dma_start(xt16r[:, 0:NI * QW16], x16r_in[0]
                             ).then_inc(s16p[0], 16)
            gpsimd.dma_start(sb8[:, B8_X23:], b8_in[:, B8_X23:]
                             ).then_inc(s_b8b, 16)
            gpsimd.memset(junk[:], 0.0).then_inc(s_mi, 1)
            gpsimd.wait_ge(s_r, 2)
            gpsimd.partition_all_reduce(rall[:], r1[:], channels=N_BINS,
                                        reduce_op=bass_isa.ReduceOp.max
                                        ).then_inc(s_g, 1)
            gpsimd.wait_ge(s_db, 1)
            gpsimd.dma_start(out[42:], db[42:]).then_inc(s_out2, 16)
            if not NO_OUT_WAIT:
                gpsimd.wait_ge(s_out2, 16)

        @block.vector
        def _(vector):
            # A: add squares, then (descale, clamp) fused 2-op
            vector.wait_ge(s_a, 1)
            vector.tensor_add(m2[:SPLIT_BIN], sq0[:], sq1[:])
            vector.drain()
            vector.tensor_scalar(m2[:SPLIT_BIN], m2[:SPLIT_BIN],
                                 cn[0:SPLIT_BIN, 2:3], AMIN2,
                                 mybir.AluOpType.mult, mybir.AluOpType.max)
            vector.drain().then_inc(s_vA, 1)
            vector.tensor_reduce(r1[:SPLIT_BIN],
                                 m2[:SPLIT_BIN].rearrange("p (i f) -> p i f", i=NI),
                                 axis=mybir.AxisListType.X, op=mybir.AluOpType.max)
            vector.drain().then_inc(s_r, 1)
            # B: m2 = re^2 + im^2 (descaled via ACT scale), clamp, reduce
            vector.wait_ge(s_a, 2)
            vector.tensor_add(m2[SPLIT_BIN:], m2[SPLIT_BIN:], tmpB[SPLIT_BIN:])
            vector.drain()
            vector.tensor_scalar_max(m2[SPLIT_BIN:], m2[SPLIT_BIN:], AMIN2)
            vector.drain().then_inc(s_vB, 1)
            vector.tensor_reduce(r1[SPLIT_BIN:],
                                 m2[SPLIT_BIN:].rearrange("p (i f) -> p i f", i=NI),
                                 axis=mybir.AxisListType.X, op=mybir.AluOpType.max)
            vector.drain().then_inc(s_r, 1)
            vector.wait_ge(s_lnr, 1)
            for i in range(NI):
                vector.tensor_scalar(db[:, i * T:(i + 1) * T],
                                     lnm[:, i * T:(i + 1) * T],
                                     lnr[:, i:i + 1], float(DB_SCALE),
                                     mybir.AluOpType.subtract,
                                     mybir.AluOpType.mult)
            vector.drain().then_inc(s_db, 1)

        @block.tensor
        def _(tensor):
            # HAM warmup: continuous PE activity during the input-DMA wait.
            for _ in range(5):
                tensor.matmul(psW[:], lhsT=junk[:, :P], rhs=junk[:, :504],
                              start=True, stop=True)
            for _ in range(2):
                tensor.matmul(psW[:, :252], lhsT=junk[:, :P], rhs=junk[:, :252],
                              start=True, stop=True)

            waited = set()

            def need(sem):
                if id(sem) not in waited:
                    tensor.wait_ge(sem, 16)
                    waited.add(id(sem))

            na = 0

            def mm16(c):
                nonlocal na
                j = CH16.index(c)
                tensor.matmul(psAv[:], lhsT=w16t[:, j * P:(j + 1) * P],
                              rhs=rhs16(c), start=(na == 0), stop=False,
                              skip_group_check=True)
                na += 1

            def mmDR(pair):
                nonlocal na
                j = PAIRS.index(pair)
                wv = wv8a[:, j] if j < N01 else wv8b[:, j - N01]
                tensor.matmul(psAv[:], lhsT=wv, rhs=rhs8(pair),
                              start=False, stop=(na == 16 + NPAIR - 1),
                              perf_mode=DR, skip_group_check=True)
                na += 1
                if na == 16 + NPAIR:
                    tensor.drain().then_inc(s_pe, 1)

            # segA0: phase-0 fp16 chunks
            need(s_b16a)
            for c in (56, 60, 64, 68):
                mm16(c)
            # segDR01: fp8 pairs over phases (0,1)
            need(s_x8a), need(s_w8a)
            for k, pair in enumerate(PAIRS01):
                if k == W8S1:
                    need(s_w8b)
                mmDR(pair)
            # segA1-A3: remaining fp16 chunks by phase
            need(s_b16b), need(s16p[0])
            for c in (57, 61, 65, 69):
                mm16(c)
            need(s16p[1])
            for c in (58, 62, 66, 70):
                mm16(c)
            need(s16p[2])
            for c in (59, 63, 67, 71):
                mm16(c)
            # segDR23: fp8 pairs over phases (2,3); closes the A chain
            need(s_b8b)
            for pair in PAIRS23:
                mmDR(pair)
            # segB: all of group B at the end; the A epilogue overlaps this
            for jb, c in enumerate(CHB):
                tensor.matmul(psBv[:], lhsT=wbt[:, jb * 64:(jb + 1) * 64],
                              rhs=rhs16(c), start=(jb == 0),
                              stop=(jb == len(CHB) - 1), skip_group_check=True)
            tensor.drain().then_inc(s_pe, 1)

    nc.compile()
    return nc


def pack_x(x):
    """x [B, 64000] f32 -> per-core blobs (b16, x16r, b8)."""
    xp = np.pad(np.asarray(x, dtype=np.float32), ((0, 0), (PAD, PAD)))
    # column-major with phase-deinterleave: x_cm[b, p, r, q] = xp[b, (4q+r)*128+p]
    x_cm = xp.reshape(B, FW // 4, 4, P).transpose(0, 3, 2, 1)  # [B,128,4,157]
    packs = []
    for core in range(N_CORES):
        blk = x_cm[core * NI:(core + 1) * NI]            # [NI, 128, 4, 157]
        t = blk.transpose(2, 1, 0, 3)                    # [4(r), 128, NI, 157]
        t16 = t[:, :, :, Q16LO:Q16HI].astype(NP16)       # [4, 128, NI, 129]
        b16 = np.zeros((P, B16_END), NP16)
        b16[:, B16_X0:B16_X0 + NI * QW16] = t16[0].reshape(P, NI * QW16)
        b16[:, B16_W16:B16_W16 + 16 * P] = W16
        b16[:, B16_WB:] = WB
        x16r = np.ascontiguousarray(t16[1:].reshape(3, P, NI * QW16))
        t8 = t.astype(NP8)                               # [4, 128, NI, 157]
        b8 = np.zeros((P, B8_END), NP8)
        b8[:, B8_X01:B8_W01] = t8[0:2].transpose(1, 0, 2, 3).reshape(P, -1)
        b8[:, B8_W01:B8_W01 + N01 * 2 * P] = W8[:, :N01 * 2 * P]
        b8[:, B8_X23:B8_X23 + 2 * NI * QW] = \
            t8[2:4].transpose(1, 0, 2, 3).reshape(P, -1)
        b8[:, B8_W23:] = W8[:, N01 * 2 * P:]
        packs.append((b16, x16r, b8))
    return packs


_PROGRAM = None


def _get_program():
    global _PROGRAM
    if _PROGRAM is None:
        _PROGRAM = build_program()
    return _PROGRAM


def run(x, **spmd_kwargs):
    """Run on 8 NeuronCores; returns (output [32, 84, 126] f32, results)."""
    nc = _get_program()
    packs = pack_x(x)
    in_maps = [{"b16_in": packs[i][0], "x16r_in": packs[i][1],
                "b8_in": packs[i][2], "cn_in": CN}
               for i in range(N_CORES)]
    res = run_bass_kernel_spmd(nc, in_maps, core_ids=list(range(N_CORES)),
                               **spmd_kwargs)
    out = np.concatenate(
        [res.results[i]["out"].reshape(N_BINS, NI, T).transpose(1, 0, 2)
         for i in range(N_CORES)], axis=0)
    return np.ascontiguousarray(out.astype(np.float32)), res


def kernel(x):
    return run(x)[0]


# revision 33
# speedup vs baseline: 1.0406x; 1.0406x over previous
"""CQT (constant-Q transform) + amplitude_to_db kernel for Trainium2.

Full-input contract: kernel(x) takes x [32, 64000] f32 and returns
[32, 84, 126] f32, matching:

    frames = pad(x, n_fft//2)[:, t*HOP + n]          # [B, 126, 16384]
    cr/ci  = frames @ Kr.T / Ki.T                    # [B, 84, 126]
    mag    = sqrt(cr^2 + ci^2)
    out    = amplitude_to_db(mag, ref=max per item, amin=1e-5, top_db=80)

Sharding: pure data parallelism - 4 batch items per NeuronCore on 8 cores.

Per-core compute layout (v2, mixed fp16 / fp8-DoubleRow):
  * One big matmul with K = n_fft = 16384 contracted in 128-row chunks;
    padded x stored column-major in SBUF so chunk c of frames^T is a strided
    AP view (HOP = 4*128).  All 4 items share each matmul via N = 504.
  * CQT kernel energy is extremely concentrated: the central 16 K-chunks
    hold 99.88% of the group-A (bins 0..63) weight energy.  Those 16 chunks
    plus all 5 group-B chunks (bins 64..83) run in fp16.  The remaining 76
    low-energy tail chunks run as 38 fp8e4m3 DoubleRow matmuls (two 128-row
    k-tiles per instruction = 2x PE throughput), with per-bin power-of-2
    weight scales to center the fp8 dynamic range.  Measured dB-domain
    rel-L2 error of this split is ~5e-3 (gate 2e-2).
  * dB epilogue: per-bin descale is folded into the ACT Square via a
    per-partition scale operand (Square(psum * 1/c_k) = m2 / c_k^2), then
    add re^2+im^2 halves, clamp at amin^2, ACT Ln, per-item max via
    free-dim reduce + GpSimd partition all-reduce, and
    out = (ln(m2c) - ln(ref2c)) * 10/ln(10).
"""

import os
import numpy as np
import ml_dtypes

import concourse.bass as bass
import concourse.mybir as mybir
from concourse import bacc
from concourse import bass_isa
from concourse.bass_utils import run_bass_kernel_spmd

# ---- problem constants (hardcoded; must match the reference) ----
SR = 22050
HOP = 512
N_BINS = 84
BPO = 12
FMIN = 32.70319566257483
AMIN = 1e-5
TOP_DB = 80.0
B = 32
N_SAMP = 64000
N_CORES = 8
NI = B // N_CORES            # items per core = 4
T = 1 + N_SAMP // HOP        # 126 frames
DB_SCALE = 10.0 / np.log(10.0)
P = 128

SPLIT_BIN = 64               # group A: bins [0,64), group B: bins [64,84)
NB_BINS = N_BINS - SPLIT_BIN  # 20

# if "1", the block does not wait for the output DMA completion semaphores;
# the framework postamble (all-engine barrier + sem-reset storm, ~7us) then
# overlaps the in-flight output DMA instead of serializing after it.
NO_OUT_WAIT = os.environ.get("CQT_NO_OUT_WAIT", "1") == "1"


def _build_cqt_kernels():
    """Same construction as the reference (nnAudio-style direct CQT bank)."""
    Q = 1.0 / (2.0 ** (1.0 / BPO) - 1.0)
    freqs = FMIN * 2.0 ** (np.arange(N_BINS) / BPO)
    lengths = np.ceil(Q * SR / freqs).astype(int)
    n_fft = int(2 ** np.ceil(np.log2(lengths.max())))
    K = np.zeros((N_BINS, n_fft), dtype=np.complex128)
    for k in range(N_BINS):
        L = int(lengths[k])
        t = np.arange(L) - (L - 1) / 2.0
        kern = np.hanning(L) * np.exp(2j * np.pi * freqs[k] * t / SR)
        kern /= np.abs(kern).sum()
        kern /= np.sqrt(L)
        s = (n_fft - L) // 2
        K[k, s:s + L] = kern
    return K.real.astype(np.float32), K.imag.astype(np.float32), n_fft


Kr, Ki, N_FFT = _build_cqt_kernels()
PAD = N_FFT // 2
FW = (N_SAMP + 2 * PAD) // P      # 628 free-dim width of column-major xp
QW = FW // 4                      # 157
NT = NI * T                       # 504
assert (N_SAMP + 2 * PAD) % P == 0 and HOP == 4 * P

# per-bin power-of-2 scale so scaled |w| peaks near 112 (fp8e4m3 max 240)
_WMAX = np.maximum(np.abs(Kr).max(1), np.abs(Ki).max(1))
BIN_SCALE = 2.0 ** np.floor(np.log2(224.0 / _WMAX / 2.0))

# ---- chunk geometry ----
# group A support: chunks [19, 109); central fp16 window [56, 72)
F0, F1 = 56, 72
CH16 = [56, 60, 64, 68, 57, 61, 65, 69, 58, 62, 66, 70, 59, 63, 67, 71]
assert sorted(CH16) == list(range(F0, F1))
# group B support: chunks [62, 67); B runs at the very end
CHB = [62, 63, 64, 65, 66]
# fp8 DR pairs (c, c+1), c even: left tail [18,56), right tail [72,110);
# chunks 18 and 109 are zero-padded (outside the true support [19,109)).
_LEFT = [(c, c + 1) for c in range(18, 56, 2)]
_RIGHT = [(c, c + 1) for c in range(72, 110, 2)]
_ALLP = _LEFT + _RIGHT
PAIRS01 = [p for p in _ALLP if p[0] % 4 == 0]   # phases (0,1)
PAIRS23 = [p for p in _ALLP if p[0] % 4 == 2]   # phases (2,3)
PAIRS = PAIRS01 + PAIRS23                        # weight-pack order
NPAIR = len(PAIRS)
assert NPAIR == 19 + 19

# fp8 weight slabs (pack order = consumption order)
W8_SLAB = [6, len(PAIRS01) - 6, len(PAIRS23)]    # 6 / 13 / 19 pairs
W8_OFF = [0, 6, len(PAIRS01), NPAIR]
N16 = len(CH16)

f16 = mybir.dt.float16
fp8 = mybir.dt.float8e4
f32 = mybir.dt.float32
DR = mybir.MatmulPerfMode.DoubleRow
NP16 = np.float16
NP8 = ml_dtypes.float8_e4m3


def _pack_weights():
    """psA plane layout: partitions [0:32) re bins 0..31, [32:64) im bins
    0..31, [64:96) re bins 32..63, [96:128) im bins 32..63.  All fp8 tail
    chunks have support only in bins 0..31 (longer kernels), so their
    DoubleRow stationary is M=64 targeting the low planes only."""
    KrT = (Kr * BIN_SCALE[:, None]).T   # [N_FFT, 84] scaled
    KiT = (Ki * BIN_SCALE[:, None]).T
    H = SPLIT_BIN // 2  # 32

    # verify the M=64 claim: tail chunks touch no bin >= 32
    for c in list(range(18, F0)) + list(range(F1, 110)):
        lo, hi = c * P, (c + 1) * P
        assert abs(KrT[lo:hi, H:SPLIT_BIN]).max() == 0
        assert abs(KiT[lo:hi, H:SPLIT_BIN]).max() == 0

    w16 = np.zeros((P, N16 * P), np.float32)
    for j, c in enumerate(CH16):
        sl = slice(c * P, (c + 1) * P)
        w16[:, j * P + 0 * H: j * P + 1 * H] = KrT[sl, 0:H]
        w16[:, j * P + 1 * H: j * P + 2 * H] = KiT[sl, 0:H]
        w16[:, j * P + 2 * H: j * P + 3 * H] = KrT[sl, H:SPLIT_BIN]
        w16[:, j * P + 3 * H: j * P + 4 * H] = KiT[sl, H:SPLIT_BIN]

    wb = np.zeros((P, len(CHB) * 64), np.float32)
    for j, c in enumerate(CHB):
        wb[:, j * 64: j * 64 + NB_BINS] = KrT[c * P:(c + 1) * P, SPLIT_BIN:]
        wb[:, j * 64 + 32: j * 64 + 32 + NB_BINS] = KiT[c * P:(c + 1) * P, SPLIT_BIN:]

    w8 = np.zeros((P, NPAIR * 2 * 64), np.float32)
    for j, (ca, cb) in enumerate(PAIRS):
        for ti, c in ((0, ca), (1, cb)):
            if c < 19 or c > 108:
                continue   # zero-padded phantom chunk
            base = j * 2 * 64 + ti * 64
            w8[:, base: base + H] = KrT[c * P:(c + 1) * P, 0:H]
            w8[:, base + H: base + 64] = KiT[c * P:(c + 1) * P, 0:H]
    return w16.astype(NP16), wb.astype(NP16), w8.astype(NP8)


W16, WB, W8 = _pack_weights()

# per-partition descale vectors:
# col 0: unused spare
# col 1: B layout 1/c_k (rows 0:20 re bins 64..83, 32:52 im), ACT Square scale
# col 2: A layout 1/c_k^2 (rows 0:64, bin k=p), DVE post-add descale
CN = np.ones((P, 3), np.float32)
CN[:NB_BINS, 1] = 1.0 / BIN_SCALE[SPLIT_BIN:]
CN[32:32 + NB_BINS, 1] = 1.0 / BIN_SCALE[SPLIT_BIN:]
CN[:SPLIT_BIN, 2] = 1.0 / BIN_SCALE[:SPLIT_BIN] ** 2


def build_program():
    nc = bacc.Bacc("TRN2", target_bir_lowering=False, debug=False,
                   enable_asserts=True)

    x16_in = nc.dram_tensor("x16_in", [4, P, NI * QW], f16,
                            kind="ExternalInput").ap()
    x8_in = nc.dram_tensor("x8_in", [2, P, 2 * NI * QW], fp8,
                           kind="ExternalInput").ap()
    wb_in = nc.dram_tensor("wb_in", [P, len(CHB) * 64], f16,
                           kind="ExternalInput").ap()
    w16_in = nc.dram_tensor("w16_in", [P, N16 * P], f16,
                            kind="ExternalInput").ap()
    w8_in = nc.dram_tensor("w8_in", [P, NPAIR * 2 * 64], fp8,
                           kind="ExternalInput").ap()
    cn_in = nc.dram_tensor("cn_in", [P, 3], f32, kind="ExternalInput").ap()
    out = nc.dram_tensor("out", [N_BINS, NT], f32, kind="ExternalOutput").ap()

    xt16 = nc.alloc_sbuf_tensor("xt16", [P, NI * FW], f16).ap()
    xt8 = nc.alloc_sbuf_tensor("xt8", [P, NI * FW], fp8).ap()
    wbt = nc.alloc_sbuf_tensor("wbt", [P, len(CHB) * 64], f16).ap()
    w16t = nc.alloc_sbuf_tensor("w16t", [P, N16 * P], f16).ap()
    w8t = nc.alloc_sbuf_tensor("w8t", [P, NPAIR * 2 * 64], fp8).ap()
    cn = nc.alloc_sbuf_tensor("cn", [P, 3], f32).ap()
    junk = nc.alloc_sbuf_tensor("junk", [P, 512], f16).ap()
    sq0 = nc.alloc_sbuf_tensor("sq0", [SPLIT_BIN, NT], f32).ap()
    sq1 = nc.alloc_sbuf_tensor("sq1", [SPLIT_BIN, NT], f32).ap()
    tmpB = nc.alloc_sbuf_tensor("tmpB", [N_BINS, NT], f32).ap()
    m2 = nc.alloc_sbuf_tensor("m2", [N_BINS, NT], f32).ap()
    lnm = nc.alloc_sbuf_tensor("lnm", [N_BINS, NT], f32).ap()
    r1 = nc.alloc_sbuf_tensor("r1", [N_BINS, NI], f32).ap()
    rall = nc.alloc_sbuf_tensor("rall", [N_BINS, NI], f32).ap()
    lnr = nc.alloc_sbuf_tensor("lnr", [N_BINS, NI], f32).ap()
    db = nc.alloc_sbuf_tensor("db", [N_BINS, NT], f32).ap()
    lnwarm = nc.alloc_sbuf_tensor("lnwarm", [1, 2], f32).ap()

    psW = nc.alloc_psum_tensor("psW", [P, NT], f32).ap()
    psA = nc.alloc_psum_tensor("psA", [P, NT], f32).ap()
    psB = nc.alloc_psum_tensor("psB", [64, NT], f32).ap()

    # one semaphore per DMA (completion order across HW queues is not
    # guaranteed on a shared counter)
    s16 = [nc.alloc_semaphore(f"s16_{r}") for r in range(4)]
    s8 = [nc.alloc_semaphore(f"s8_{h}") for h in range(2)]
    s_wb = nc.alloc_semaphore("s_wb")
    s_w16 = [nc.alloc_semaphore(f"s_w16_{i}") for i in range(2)]
    s_w8 = [nc.alloc_semaphore(f"s_w8_{i}") for i in range(3)]
    s_ic = nc.alloc_semaphore("s_ic")
    s_mi = nc.alloc_semaphore("s_mi")
    s_pe = nc.alloc_semaphore("s_pe")     # 1 = psA final, 2 = psB final
    s_a = nc.alloc_semaphore("s_a")       # ACT squares done (1=B, 2=A)
    s_vB = nc.alloc_semaphore("s_vB")     # m2c B ready
    s_vA = nc.alloc_semaphore("s_vA")     # m2c A ready
    s_r = nc.alloc_semaphore("s_r")       # r1 halves done (1=B, 2=A)
    s_g = nc.alloc_semaphore("s_g")       # allreduce done
    s_lnr = nc.alloc_semaphore("s_lnr")   # lnr (and lnm) ready
    s_db = nc.alloc_semaphore("s_db")     # db ready for output
    s_out = nc.alloc_semaphore("s_out")
    s_out2 = nc.alloc_semaphore("s_out2")

    xv16 = xt16.rearrange("p (r i q) -> p r i q", r=4, i=NI)
    # fp8 pair view: phase-pair rp in {0,1}, k-tile j in {0,1}
    xv8 = xt8.rearrange("p (rp j i q) -> p rp j i q", rp=2, j=2, i=NI)
    wv8 = w8t.rearrange("p (j two m) -> p j two m", two=2, m=64)
    psAv = psA.rearrange("p (i t) -> p i t", i=NI)
    psBv = psB.rearrange("p (i t) -> p i t", i=NI)

    def rhs16(c):
        r, q0 = c % 4, c // 4
        return xv16[:, r, :, q0: q0 + T]

    def rhs8(pair):
        c = pair[0]
        rp, q0 = c // 2 % 2, c // 4
        return xv8[:, rp, :, :, q0: q0 + T]

    Ln = mybir.ActivationFunctionType.Ln
    Square = mybir.ActivationFunctionType.Square
    AMIN2 = float(AMIN) ** 2

    with nc.Block(no_gpsimd_drain=True) as block:

        @block.sync
        def _(sync):
            # issues in global consumption order across the three DMA engines
            sync.dma_start(xt16[:, 0:NI * QW], x16_in[0]).then_inc(s16[0], 16)
            sync.dma_start(xt8[:, 0:2 * NI * QW], x8_in[0]).then_inc(s8[0], 16)
            sync.dma_start(xt16[:, NI * QW:2 * NI * QW], x16_in[1]
                           ).then_inc(s16[1], 16)
            sync.dma_start(xt16[:, 3 * NI * QW:], x16_in[3]).then_inc(s16[3], 16)
            sync.wait_ge(s_db, 1)
            sync.dma_start(out[0:42], db[0:42]).then_inc(s_out, 16)
            if not NO_OUT_WAIT:
                sync.wait_ge(s_out, 16)

        @block.scalar
        def _(scalar):
            scalar.dma_start(wbt[:], wb_in).then_inc(s_wb, 16)
            scalar.dma_start(w8t[:, :W8_OFF[1] * 128],
                             w8_in[:, :W8_OFF[1] * 128]).then_inc(s_w8[0], 16)
            scalar.dma_start(w16t[:, 4 * P:], w16_in[:, 4 * P:]
                             ).then_inc(s_w16[1], 16)
            scalar.dma_start(xt8[:, 2 * NI * QW:], x8_in[1]).then_inc(s8[1], 16)
            scalar.dma_start(w8t[:, W8_OFF[2] * 128:],
                             w8_in[:, W8_OFF[2] * 128:]).then_inc(s_w8[2], 16)
            # preload BOTH act table slots (Ln set + Square set)
            scalar.activation(lnwarm[:, 0:1], nc.const_aps.tensor(1.0, (1, 1)), Ln)
            scalar.activation(lnwarm[:, 1:2], nc.const_aps.tensor(1.0, (1, 1)),
                              Square)
            # A epilogue first (A chain closes before the B matmuls run)
            scalar.wait_ge(s_pe, 1)
            # psA planes: [0:32) re-lo, [32:64) im-lo, [64:96) re-hi,
            # [96:128) im-hi; remap so sq0 = re^2, sq1 = im^2 by bin
            scalar.activation(sq0[0:32], psA[0:32], Square)
            scalar.activation(sq1[0:32], psA[32:64], Square)
            scalar.activation(sq0[32:64], psA[64:96], Square)
            scalar.activation(sq1[32:64], psA[96:128], Square).then_inc(s_a)
            # B epilogue
            scalar.wait_ge(s_pe, 2)
            scalar.wait_ge(s_ic, 16)
            scalar.activation(m2[SPLIT_BIN:], psB[0:NB_BINS], Square,
                              scale=cn[0:NB_BINS, 1:2])
            scalar.activation(tmpB[SPLIT_BIN:], psB[32:32 + NB_BINS], Square,
                              scale=cn[32:32 + NB_BINS, 1:2]).then_inc(s_a)
            scalar.wait_ge(s_vA, 1)
            scalar.activation(lnm[:SPLIT_BIN], m2[:SPLIT_BIN], Ln)
            scalar.wait_ge(s_vB, 1)
            scalar.activation(lnm[SPLIT_BIN:], m2[SPLIT_BIN:], Ln)
            scalar.wait_ge(s_g, 1)
            scalar.activation(lnr[:], rall[:], Ln).then_inc(s_lnr)

        @block.gpsimd
        def _(gpsimd):
            gpsimd.dma_start(w16t[:, :4 * P], w16_in[:, :4 * P]
                             ).then_inc(s_w16[0], 16)
            gpsimd.dma_start(
                w8t[:, W8_OFF[1] * 128:W8_OFF[2] * 128],
                w8_in[:, W8_OFF[1] * 128:W8_OFF[2] * 128]
            ).then_inc(s_w8[1], 16)
            gpsimd.dma_start(xt16[:, 2 * NI * QW:3 * NI * QW], x16_in[2]
                             ).then_inc(s16[2], 16)
            gpsimd.dma_start(cn[:], cn_in).then_inc(s_ic, 16)
            gpsimd.memset(junk[:], 0.0).then_inc(s_mi, 1)
            gpsimd.wait_ge(s_r, 2)
            gpsimd.partition_all_reduce(rall[:], r1[:], channels=N_BINS,
                                        reduce_op=bass_isa.ReduceOp.max
                                        ).then_inc(s_g, 1)
            gpsimd.wait_ge(s_db, 1)
            gpsimd.dma_start(out[42:], db[42:]).then_inc(s_out2, 16)
            if not NO_OUT_WAIT:
                gpsimd.wait_ge(s_out2, 16)

        @block.vector
        def _(vector):
            # A: add raw squares, then (descale, clamp) fused 2-op
            vector.wait_ge(s_a, 1)
            vector.tensor_add(m2[:SPLIT_BIN], sq0[:], sq1[:])
            vector.drain()
            vector.tensor_scalar(m2[:SPLIT_BIN], m2[:SPLIT_BIN],
                                 cn[0:SPLIT_BIN, 2:3], AMIN2,
                                 mybir.AluOpType.mult, mybir.AluOpType.max)
            vector.drain().then_inc(s_vA, 1)
            vector.tensor_reduce(r1[:SPLIT_BIN],
                                 m2[:SPLIT_BIN].rearrange("p (i f) -> p i f", i=NI),
                                 axis=mybir.AxisListType.X, op=mybir.AluOpType.max)
            vector.drain().then_inc(s_r, 1)
            # B: m2 = re^2 + im^2 (descaled via ACT scale), clamp, reduce
            vector.wait_ge(s_a, 2)
            vector.tensor_add(m2[SPLIT_BIN:], m2[SPLIT_BIN:], tmpB[SPLIT_BIN:])
            vector.drain()
            vector.tensor_scalar_max(m2[SPLIT_BIN:], m2[SPLIT_BIN:], AMIN2)
            vector.drain().then_inc(s_vB, 1)
            vector.tensor_reduce(r1[SPLIT_BIN:],
                                 m2[SPLIT_BIN:].rearrange("p (i f) -> p i f", i=NI),
                                 axis=mybir.AxisListType.X, op=mybir.AluOpType.max)
            vector.drain().then_inc(s_r, 1)
            vector.wait_ge(s_lnr, 1)
            for i in range(NI):
                vector.tensor_scalar(db[:, i * T:(i + 1) * T],
                                     lnm[:, i * T:(i + 1) * T],
                                     lnr[:, i:i + 1], float(DB_SCALE),
                                     mybir.AluOpType.subtract,
                                     mybir.AluOpType.mult)
            vector.drain().then_inc(s_db, 1)

        @block.tensor
        def _(tensor):
            # HAM warmup: continuous PE activity during the input-DMA wait.
            # No wait on the junk memset - the first warmup matmuls may read
            # garbage; psW is never consumed.
            for _ in range(6):
                tensor.matmul(psW[:], lhsT=junk[:, :P], rhs=junk[:, :504],
                              start=True, stop=True)
            for _ in range(2):
                tensor.matmul(psW[:, :252], lhsT=junk[:, :P], rhs=junk[:, :252],
                              start=True, stop=True)

            waited = set()

            def need(sem):
                if id(sem) not in waited:
                    tensor.wait_ge(sem, 16)
                    waited.add(id(sem))

            na = 0

            def mm16(c):
                nonlocal na
                j = CH16.index(c)
                tensor.matmul(psAv[:], lhsT=w16t[:, j * P:(j + 1) * P],
                              rhs=rhs16(c), start=(na == 0), stop=False,
                              skip_group_check=True)
                na += 1

            def mmDR(pair):
                nonlocal na
                j = PAIRS.index(pair)
                tensor.matmul(psAv[0:SPLIT_BIN], lhsT=wv8[:, j], rhs=rhs8(pair),
                              start=False, stop=(na == N16 + NPAIR - 1),
                              perf_mode=DR, skip_group_check=True)
                na += 1
                if na == N16 + NPAIR:
                    tensor.drain().then_inc(s_pe, 1)

            # seg1: phase 0 fp16
            need(s_wb), need(s16[0]), need(s_w16[0])
            for c in (56, 60, 64, 68):
                mm16(c)
            # seg2: fp8 DR pairs, phases (0,1) - x8p01 + first w8 slabs land
            # before x16 phase 1 does
            need(s8[0]), need(s_w8[0])
            for k, pair in enumerate(PAIRS01):
                if k == W8_SLAB[0]:
                    need(s_w8[1])
                mmDR(pair)
            # seg3: phase 1 fp16
            need(s16[1]), need(s_w16[1])
            for c in (57, 61, 65, 69):
                mm16(c)
            # seg4: phase 2 fp16
            need(s16[2]), need(s_w16[1])
            for c in (58, 62, 66, 70):
                mm16(c)
            # seg5: phase 3 fp16
            need(s16[3])
            for c in (59, 63, 67, 71):
                mm16(c)
            # seg6: fp8 DR pairs, phases (2,3) (closes the A chain)
            need(s8[1]), need(s_w8[2])
            for pair in PAIRS23:
                mmDR(pair)
            # segB: all of group B at the end; the A epilogue overlaps this
            for jb, c in enumerate(CHB):
                tensor.matmul(psBv[:], lhsT=wbt[:, jb * 64:(jb + 1) * 64],
                              rhs=rhs16(c), start=(jb == 0),
                              stop=(jb == len(CHB) - 1), skip_group_check=True)
            tensor.drain().then_inc(s_pe, 1)

    nc.compile()
    return nc


def pack_x(x):
    """x [B, 64000] f32 -> per-core (x16 [4,P,NI*QW] f16, x8 [2,P,2*NI*QW] e4m3)."""
    xp = np.pad(np.asarray(x, dtype=np.float32), ((0, 0), (PAD, PAD)))
    # column-major with phase-deinterleave: x_cm[b, p, r, q] = xp[b, (4q+r)*128+p]
    x_cm = xp.reshape(B, FW // 4, 4, P).transpose(0, 3, 2, 1)  # [B,128,4,157]
    packs = []
    for core in range(N_CORES):
        blk = x_cm[core * NI:(core + 1) * NI]            # [NI, 128, 4, 157]
        t = blk.transpose(2, 1, 0, 3)                    # [4, 128, NI, 157]
        p16 = np.ascontiguousarray(t.reshape(4, P, NI * QW)).astype(NP16)
        p8 = np.ascontiguousarray(t.reshape(2, 2, P, NI * QW)
                                  .transpose(0, 2, 1, 3)
                                  .reshape(2, P, 2 * NI * QW)).astype(NP8)
        packs.append((p16, p8))
    return packs


_PROGRAM = None


def _get_program():
    global _PROGRAM
    if _PROGRAM is None:
        _PROGRAM = build_program()
    return _PROGRAM


def run(x, **spmd_kwargs):
    """Run on 8 NeuronCores; returns (output [32, 84, 126] f32, results)."""
    nc = _get_program()
    packs = pack_x(x)
    in_maps = [{"x16_in": packs[i][0], "x8_in": packs[i][1],
                "wb_in": WB, "w16_in": W16, "w8_in": W8, "cn_in": CN}
               for i in range(N_CORES)]
    res = run_bass_kernel_spmd(nc, in_maps, core_ids=list(range(N_CORES)),
                               **spmd_kwargs)
    out = np.concatenate(
        [res.results[i]["out"].reshape(N_BINS, NI, T).transpose(1, 0, 2)
         for i in range(N_CORES)], axis=0)
    return np.ascontiguousarray(out.astype(np.float32)), res


def kernel(x):
    return run(x)[0]


# revision 35
# speedup vs baseline: 1.2274x; 1.1795x over previous
"""CQT (constant-Q transform) + amplitude_to_db kernel for Trainium2.

Full-input contract: kernel(x) takes x [32, 64000] f32 and returns
[32, 84, 126] f32, matching:

    frames = pad(x, n_fft//2)[:, t*HOP + n]          # [B, 126, 16384]
    cr/ci  = frames @ Kr.T / Ki.T                    # [B, 84, 126]
    mag    = sqrt(cr^2 + ci^2)
    out    = amplitude_to_db(mag, ref=max per item, amin=1e-5, top_db=80)

Sharding: pure data parallelism - 4 batch items per NeuronCore on 8 cores.

Per-core compute layout (v2, mixed fp16 / fp8-DoubleRow):
  * One big matmul with K = n_fft = 16384 contracted in 128-row chunks;
    padded x stored column-major in SBUF so chunk c of frames^T is a strided
    AP view (HOP = 4*128).  All 4 items share each matmul via N = 504.
  * CQT kernel energy is extremely concentrated: the central 16 K-chunks
    hold 99.88% of the group-A (bins 0..63) weight energy.  Those 16 chunks
    plus all 5 group-B chunks (bins 64..83) run in fp16.  The remaining 76
    low-energy tail chunks run as 38 fp8e4m3 DoubleRow matmuls (two 128-row
    k-tiles per instruction = 2x PE throughput), with per-bin power-of-2
    weight scales to center the fp8 dynamic range.  Measured dB-domain
    rel-L2 error of this split is ~5e-3 (gate 2e-2).
  * dB epilogue: per-bin descale is folded into the ACT Square via a
    per-partition scale operand (Square(psum * 1/c_k) = m2 / c_k^2), then
    add re^2+im^2 halves, clamp at amin^2, ACT Ln, per-item max via
    free-dim reduce + GpSimd partition all-reduce, and
    out = (ln(m2c) - ln(ref2c)) * 10/ln(10).
"""

import os
import numpy as np
import ml_dtypes

import concourse.bass as bass
import concourse.mybir as mybir
from concourse import bacc
from concourse import bass_isa
from concourse.bass_utils import run_bass_kernel_spmd

# ---- problem constants (hardcoded; must match the reference) ----
SR = 22050
HOP = 512
N_BINS = 84
BPO = 12
FMIN = 32.70319566257483
AMIN = 1e-5
TOP_DB = 80.0
B = 32
N_SAMP = 64000
N_CORES = 8
NI = B // N_CORES            # items per core = 4
T = 1 + N_SAMP // HOP        # 126 frames
DB_SCALE = 10.0 / np.log(10.0)
P = 128

SPLIT_BIN = 64               # group A: bins [0,64), group B: bins [64,84)
NB_BINS = N_BINS - SPLIT_BIN  # 20

# if "1", the block does not wait for the output DMA completion semaphores;
# the framework postamble (all-engine barrier + sem-reset storm, ~7us) then
# overlaps the in-flight output DMA instead of serializing after it.
NO_OUT_WAIT = os.environ.get("CQT_NO_OUT_WAIT", "1") == "1"


def _build_cqt_kernels():
    """Same construction as the reference (nnAudio-style direct CQT bank)."""
    Q = 1.0 / (2.0 ** (1.0 / BPO) - 1.0)
    freqs = FMIN * 2.0 ** (np.arange(N_BINS) / BPO)
    lengths = np.ceil(Q * SR / freqs).astype(int)
    n_fft = int(2 ** np.ceil(np.log2(lengths.max())))
    K = np.zeros((N_BINS, n_fft), dtype=np.complex128)
    for k in range(N_BINS):
        L = int(lengths[k])
        t = np.arange(L) - (L - 1) / 2.0
        kern = np.hanning(L) * np.exp(2j * np.pi * freqs[k] * t / SR)
        kern /= np.abs(kern).sum()
        kern /= np.sqrt(L)
        s = (n_fft - L) // 2
        K[k, s:s + L] = kern
    return K.real.astype(np.float32), K.imag.astype(np.float32), n_fft


Kr, Ki, N_FFT = _build_cqt_kernels()
PAD = N_FFT // 2
FW = (N_SAMP + 2 * PAD) // P      # 628 free-dim width of column-major xp
QW = FW // 4                      # 157
NT = NI * T                       # 504
assert (N_SAMP + 2 * PAD) % P == 0 and HOP == 4 * P

# per-bin power-of-2 scale so scaled |w| peaks near 112 (fp8e4m3 max 240)
_WMAX = np.maximum(np.abs(Kr).max(1), np.abs(Ki).max(1))
BIN_SCALE = 2.0 ** np.floor(np.log2(224.0 / _WMAX / 2.0))

# ---- chunk geometry ----
# group A support: chunks [19, 109); central fp16 window [56, 72)
F0, F1 = 56, 72
CH16 = [56, 60, 64, 68, 57, 61, 65, 69, 58, 62, 66, 70, 59, 63, 67, 71]
assert sorted(CH16) == list(range(F0, F1))
# group B support: chunks [62, 67); B runs at the very end
CHB = [62, 63, 64, 65, 66]
# fp8 DR pairs (c, c+1), c even: left tail [18,56), right tail [72,110);
# chunks 18 and 109 are zero-padded (outside the true support [19,109)).
_LEFT = [(c, c + 1) for c in range(18, 56, 2)]
_RIGHT = [(c, c + 1) for c in range(72, 110, 2)]
_ALLP = _LEFT + _RIGHT
PAIRS01 = [p for p in _ALLP if p[0] % 4 == 0]   # phases (0,1)
PAIRS23 = [p for p in _ALLP if p[0] % 4 == 2]   # phases (2,3)
PAIRS = PAIRS01 + PAIRS23                        # weight-pack order
NPAIR = len(PAIRS)
assert NPAIR == 19 + 19

# fp8 weight slabs (pack order = consumption order)
W8_SLAB = [6, len(PAIRS01) - 6, len(PAIRS23)]    # 6 / 13 / 19 pairs
W8_OFF = [0, 6, len(PAIRS01), NPAIR]
N16 = len(CH16)

f16 = mybir.dt.float16
fp8 = mybir.dt.float8e4
f32 = mybir.dt.float32
DR = mybir.MatmulPerfMode.DoubleRow
NP16 = np.float16
NP8 = ml_dtypes.float8_e4m3


def _pack_weights():
    """psA plane layout: partitions [0:32) re bins 0..31, [32:64) im bins
    0..31, [64:96) re bins 32..63, [96:128) im bins 32..63.  All fp8 tail
    chunks have support only in bins 0..31 (longer kernels), so their
    DoubleRow stationary is M=64 targeting the low planes only."""
    KrT = (Kr * BIN_SCALE[:, None]).T   # [N_FFT, 84] scaled
    KiT = (Ki * BIN_SCALE[:, None]).T
    H = SPLIT_BIN // 2  # 32

    # verify the M=64 claim: tail chunks touch no bin >= 32
    for c in list(range(18, F0)) + list(range(F1, 110)):
        lo, hi = c * P, (c + 1) * P
        assert abs(KrT[lo:hi, H:SPLIT_BIN]).max() == 0
        assert abs(KiT[lo:hi, H:SPLIT_BIN]).max() == 0

    w16 = np.zeros((P, N16 * P), np.float32)
    for j, c in enumerate(CH16):
        sl = slice(c * P, (c + 1) * P)
        w16[:, j * P + 0 * H: j * P + 1 * H] = KrT[sl, 0:H]
        w16[:, j * P + 1 * H: j * P + 2 * H] = KiT[sl, 0:H]
        w16[:, j * P + 2 * H: j * P + 3 * H] = KrT[sl, H:SPLIT_BIN]
        w16[:, j * P + 3 * H: j * P + 4 * H] = KiT[sl, H:SPLIT_BIN]

    wb = np.zeros((P, len(CHB) * 64), np.float32)
    for j, c in enumerate(CHB):
        wb[:, j * 64: j * 64 + NB_BINS] = KrT[c * P:(c + 1) * P, SPLIT_BIN:]
        wb[:, j * 64 + 32: j * 64 + 32 + NB_BINS] = KiT[c * P:(c + 1) * P, SPLIT_BIN:]

    w8 = np.zeros((P, NPAIR * 2 * 64), np.float32)
    for j, (ca, cb) in enumerate(PAIRS):
        for ti, c in ((0, ca), (1, cb)):
            if c < 19 or c > 108:
                continue   # zero-padded phantom chunk
            base = j * 2 * 64 + ti * 64
            w8[:, base: base + H] = KrT[c * P:(c + 1) * P, 0:H]
            w8[:, base + H: base + 64] = KiT[c * P:(c + 1) * P, 0:H]
    return w16.astype(NP16), wb.astype(NP16), w8.astype(NP8)


W16, WB, W8 = _pack_weights()

# per-partition descale vectors:
# col 0: unused spare
# col 1: B layout 1/c_k (rows 0:20 re bins 64..83, 32:52 im), ACT Square scale
# col 2: A layout 1/c_k^2 (rows 0:64, bin k=p), DVE post-add descale
CN = np.ones((P, 3), np.float32)
CN[:NB_BINS, 1] = 1.0 / BIN_SCALE[SPLIT_BIN:]
CN[32:32 + NB_BINS, 1] = 1.0 / BIN_SCALE[SPLIT_BIN:]
CN[:SPLIT_BIN, 2] = 1.0 / BIN_SCALE[:SPLIT_BIN] ** 2


def build_program():
    nc = bacc.Bacc("TRN2", target_bir_lowering=False, debug=False,
                   enable_asserts=True)

    x16_in = nc.dram_tensor("x16_in", [4, P, NI * QW], f16,
                            kind="ExternalInput").ap()
    x8_in = nc.dram_tensor("x8_in", [2, P, 2 * NI * QW], fp8,
                           kind="ExternalInput").ap()
    wb_in = nc.dram_tensor("wb_in", [P, len(CHB) * 64], f16,
                           kind="ExternalInput").ap()
    w16_in = nc.dram_tensor("w16_in", [P, N16 * P], f16,
                            kind="ExternalInput").ap()
    w8_in = nc.dram_tensor("w8_in", [P, NPAIR * 2 * 64], fp8,
                           kind="ExternalInput").ap()
    cn_in = nc.dram_tensor("cn_in", [P, 3], f32, kind="ExternalInput").ap()
    out = nc.dram_tensor("out", [N_BINS, NT], f32, kind="ExternalOutput").ap()

    xt16 = nc.alloc_sbuf_tensor("xt16", [P, NI * FW], f16).ap()
    xt8 = nc.alloc_sbuf_tensor("xt8", [P, NI * FW], fp8).ap()
    wbt = nc.alloc_sbuf_tensor("wbt", [P, len(CHB) * 64], f16).ap()
    w16t = nc.alloc_sbuf_tensor("w16t", [P, N16 * P], f16).ap()
    w8t = nc.alloc_sbuf_tensor("w8t", [P, NPAIR * 2 * 64], fp8).ap()
    cn = nc.alloc_sbuf_tensor("cn", [P, 3], f32).ap()
    junk = nc.alloc_sbuf_tensor("junk", [P, 512], f16).ap()
    sq0 = nc.alloc_sbuf_tensor("sq0", [SPLIT_BIN, NT], f32).ap()
    sq1 = nc.alloc_sbuf_tensor("sq1", [SPLIT_BIN, NT], f32).ap()
    tmpB = nc.alloc_sbuf_tensor("tmpB", [N_BINS, NT], f32).ap()
    m2 = nc.alloc_sbuf_tensor("m2", [N_BINS, NT], f32).ap()
    lnm = nc.alloc_sbuf_tensor("lnm", [N_BINS, NT], f32).ap()
    r1 = nc.alloc_sbuf_tensor("r1", [N_BINS, NI], f32).ap()
    rall = nc.alloc_sbuf_tensor("rall", [N_BINS, NI], f32).ap()
    lnr = nc.alloc_sbuf_tensor("lnr", [N_BINS, NI], f32).ap()
    db = nc.alloc_sbuf_tensor("db", [N_BINS, NT], f32).ap()
    lnwarm = nc.alloc_sbuf_tensor("lnwarm", [1, 2], f32).ap()

    psW = nc.alloc_psum_tensor("psW", [P, NT], f32).ap()
    psA = nc.alloc_psum_tensor("psA", [P, NT], f32).ap()
    psB = nc.alloc_psum_tensor("psB", [64, NT], f32).ap()

    # one semaphore per DMA (completion order across HW queues is not
    # guaranteed on a shared counter)
    s16 = [nc.alloc_semaphore(f"s16_{r}") for r in range(4)]
    s8 = [nc.alloc_semaphore(f"s8_{h}") for h in range(2)]
    s_wb = nc.alloc_semaphore("s_wb")
    s_w16 = [nc.alloc_semaphore(f"s_w16_{i}") for i in range(2)]
    s_w8 = [nc.alloc_semaphore(f"s_w8_{i}") for i in range(3)]
    s_ic = nc.alloc_semaphore("s_ic")
    s_mi = nc.alloc_semaphore("s_mi")
    s_pe = nc.alloc_semaphore("s_pe")     # 1 = psA final, 2 = psB final
    s_hi = nc.alloc_semaphore("s_hi")     # psA hi planes final (A16 all done)
    s_a = nc.alloc_semaphore("s_a")       # ACT squares done (1=B, 2=A)
    s_vB = nc.alloc_semaphore("s_vB")     # m2c B ready
    s_vA = nc.alloc_semaphore("s_vA")     # m2c A ready
    s_r = nc.alloc_semaphore("s_r")       # r1 halves done (1=B, 2=A)
    s_g = nc.alloc_semaphore("s_g")       # allreduce done
    s_lnr = nc.alloc_semaphore("s_lnr")   # lnr (and lnm) ready
    s_db = nc.alloc_semaphore("s_db")     # db ready for output
    s_out = nc.alloc_semaphore("s_out")
    s_out2 = nc.alloc_semaphore("s_out2")

    xv16 = xt16.rearrange("p (r i q) -> p r i q", r=4, i=NI)
    # fp8 pair view: phase-pair rp in {0,1}, k-tile j in {0,1}
    xv8 = xt8.rearrange("p (rp j i q) -> p rp j i q", rp=2, j=2, i=NI)
    wv8 = w8t.rearrange("p (j two m) -> p j two m", two=2, m=64)
    psAv = psA.rearrange("p (i t) -> p i t", i=NI)
    psBv = psB.rearrange("p (i t) -> p i t", i=NI)

    def rhs16(c):
        r, q0 = c % 4, c // 4
        return xv16[:, r, :, q0: q0 + T]

    def rhs8(pair):
        c = pair[0]
        rp, q0 = c // 2 % 2, c // 4
        return xv8[:, rp, :, :, q0: q0 + T]

    Ln = mybir.ActivationFunctionType.Ln
    Square = mybir.ActivationFunctionType.Square
    AMIN2 = float(AMIN) ** 2

    with nc.Block(no_gpsimd_drain=True) as block:

        @block.sync
        def _(sync):
            # issues in global consumption order across the three DMA engines
            sync.dma_start(xt16[:, 0:NI * QW], x16_in[0]).then_inc(s16[0], 16)
            sync.dma_start(xt8[:, 0:2 * NI * QW], x8_in[0]).then_inc(s8[0], 16)
            sync.dma_start(xt16[:, NI * QW:2 * NI * QW], x16_in[1]
                           ).then_inc(s16[1], 16)
            sync.dma_start(xt16[:, 3 * NI * QW:], x16_in[3]).then_inc(s16[3], 16)
            sync.wait_ge(s_db, 1)
            sync.dma_start(out[0:42], db[0:42]).then_inc(s_out, 16)
            if not NO_OUT_WAIT:
                sync.wait_ge(s_out, 16)

        @block.scalar
        def _(scalar):
            scalar.dma_start(wbt[:], wb_in).then_inc(s_wb, 16)
            scalar.dma_start(w8t[:, :W8_OFF[1] * 128],
                             w8_in[:, :W8_OFF[1] * 128]).then_inc(s_w8[0], 16)
            scalar.dma_start(w16t[:, 4 * P:], w16_in[:, 4 * P:]
                             ).then_inc(s_w16[1], 16)
            scalar.dma_start(xt8[:, 2 * NI * QW:], x8_in[1]).then_inc(s8[1], 16)
            scalar.dma_start(w8t[:, W8_OFF[2] * 128:],
                             w8_in[:, W8_OFF[2] * 128:]).then_inc(s_w8[2], 16)
            # preload BOTH act table slots (Ln set + Square set)
            scalar.activation(lnwarm[:, 0:1], nc.const_aps.tensor(1.0, (1, 1)), Ln)
            scalar.activation(lnwarm[:, 1:2], nc.const_aps.tensor(1.0, (1, 1)),
                              Square)
            # psA planes: [0:32) re-lo, [32:64) im-lo, [64:96) re-hi,
            # [96:128) im-hi; remap so sq0 = re^2, sq1 = im^2 by bin.
            # Hi planes are only written by the fp16 chunks, all of which
            # precede segDR23 - square them during the DR23 stream.
            scalar.wait_ge(s_hi, 1)
            scalar.activation(sq0[32:64], psA[64:96], Square)
            scalar.activation(sq1[32:64], psA[96:128], Square)
            # lo planes after the A chain closes
            scalar.wait_ge(s_pe, 1)
            scalar.activation(sq0[0:32], psA[0:32], Square)
            scalar.activation(sq1[0:32], psA[32:64], Square).then_inc(s_a)
            # B epilogue
            scalar.wait_ge(s_pe, 2)
            scalar.wait_ge(s_ic, 16)
            scalar.activation(m2[SPLIT_BIN:], psB[0:NB_BINS], Square,
                              scale=cn[0:NB_BINS, 1:2])
            scalar.activation(tmpB[SPLIT_BIN:], psB[32:32 + NB_BINS], Square,
                              scale=cn[32:32 + NB_BINS, 1:2]).then_inc(s_a)
            scalar.wait_ge(s_vA, 1)
            scalar.activation(lnm[:SPLIT_BIN], m2[:SPLIT_BIN], Ln)
            scalar.wait_ge(s_vB, 1)
            scalar.activation(lnm[SPLIT_BIN:], m2[SPLIT_BIN:], Ln)
            scalar.wait_ge(s_g, 1)
            scalar.activation(lnr[:], rall[:], Ln).then_inc(s_lnr)

        @block.gpsimd
        def _(gpsimd):
            gpsimd.dma_start(w16t[:, :4 * P], w16_in[:, :4 * P]
                             ).then_inc(s_w16[0], 16)
            gpsimd.dma_start(
                w8t[:, W8_OFF[1] * 128:W8_OFF[2] * 128],
                w8_in[:, W8_OFF[1] * 128:W8_OFF[2] * 128]
            ).then_inc(s_w8[1], 16)
            gpsimd.dma_start(xt16[:, 2 * NI * QW:3 * NI * QW], x16_in[2]
                             ).then_inc(s16[2], 16)
            gpsimd.dma_start(cn[:], cn_in).then_inc(s_ic, 16)
            gpsimd.memset(junk[:], 0.0).then_inc(s_mi, 1)
            gpsimd.wait_ge(s_r, 2)
            gpsimd.partition_all_reduce(rall[:], r1[:], channels=N_BINS,
                                        reduce_op=bass_isa.ReduceOp.max
                                        ).then_inc(s_g, 1)
            gpsimd.wait_ge(s_db, 1)
            gpsimd.dma_start(out[42:], db[42:]).then_inc(s_out2, 16)
            if not NO_OUT_WAIT:
                gpsimd.wait_ge(s_out2, 16)

        @block.vector
        def _(vector):
            # A: add raw squares, then (descale, clamp) fused 2-op
            vector.wait_ge(s_a, 1)
            vector.tensor_add(m2[:SPLIT_BIN], sq0[:], sq1[:])
            vector.drain()
            vector.tensor_scalar(m2[:SPLIT_BIN], m2[:SPLIT_BIN],
                                 cn[0:SPLIT_BIN, 2:3], AMIN2,
                                 mybir.AluOpType.mult, mybir.AluOpType.max)
            vector.drain().then_inc(s_vA, 1)
            vector.tensor_reduce(r1[:SPLIT_BIN],
                                 m2[:SPLIT_BIN].rearrange("p (i f) -> p i f", i=NI),
                                 axis=mybir.AxisListType.X, op=mybir.AluOpType.max)
            vector.drain().then_inc(s_r, 1)
            # B: m2 = re^2 + im^2 (descaled via ACT scale), clamp, reduce
            vector.wait_ge(s_a, 2)
            vector.tensor_add(m2[SPLIT_BIN:], m2[SPLIT_BIN:], tmpB[SPLIT_BIN:])
            vector.drain()
            vector.tensor_scalar_max(m2[SPLIT_BIN:], m2[SPLIT_BIN:], AMIN2)
            vector.drain().then_inc(s_vB, 1)
            vector.tensor_reduce(r1[SPLIT_BIN:],
                                 m2[SPLIT_BIN:].rearrange("p (i f) -> p i f", i=NI),
                                 axis=mybir.AxisListType.X, op=mybir.AluOpType.max)
            vector.drain().then_inc(s_r, 1)
            vector.wait_ge(s_lnr, 1)
            for i in range(NI):
                vector.tensor_scalar(db[:, i * T:(i + 1) * T],
                                     lnm[:, i * T:(i + 1) * T],
                                     lnr[:, i:i + 1], float(DB_SCALE),
                                     mybir.AluOpType.subtract,
                                     mybir.AluOpType.mult)
            vector.drain().then_inc(s_db, 1)

        @block.tensor
        def _(tensor):
            # HAM warmup: continuous PE activity during the input-DMA wait.
            # No wait on the junk memset - the first warmup matmuls may read
            # garbage; psW is never consumed.
            for _ in range(6):
                tensor.matmul(psW[:], lhsT=junk[:, :P], rhs=junk[:, :504],
                              start=True, stop=True)
            for _ in range(2):
                tensor.matmul(psW[:, :252], lhsT=junk[:, :P], rhs=junk[:, :252],
                              start=True, stop=True)

            waited = set()

            def need(sem):
                if id(sem) not in waited:
                    tensor.wait_ge(sem, 16)
                    waited.add(id(sem))

            na = 0

            def mm16(c):
                nonlocal na
                j = CH16.index(c)
                tensor.matmul(psAv[:], lhsT=w16t[:, j * P:(j + 1) * P],
                              rhs=rhs16(c), start=(na == 0), stop=False,
                              skip_group_check=True)
                na += 1

            def mmDR(pair):
                nonlocal na
                j = PAIRS.index(pair)
                inst = tensor.matmul(psAv[0:SPLIT_BIN], lhsT=wv8[:, j],
                                     rhs=rhs8(pair), start=False,
                                     stop=(na == N16 + NPAIR - 1),
                                     perf_mode=DR, skip_group_check=True)
                na += 1
                if na == N16 + NPAIR:
                    tensor.drain().then_inc(s_pe, 1)
                return inst

            # seg1: phase 0 fp16
            need(s_wb), need(s16[0]), need(s_w16[0])
            for c in (56, 60, 64, 68):
                mm16(c)
            # seg2: fp8 DR pairs, phases (0,1) - x8p01 + first w8 slabs land
            # before x16 phase 1 does
            need(s8[0]), need(s_w8[0])
            for k, pair in enumerate(PAIRS01):
                if k == W8_SLAB[0]:
                    need(s_w8[1])
                mmDR(pair)
            # seg3: phase 1 fp16
            need(s16[1]), need(s_w16[1])
            for c in (57, 61, 65, 69):
                mm16(c)
            # seg4: phase 2 fp16
            need(s16[2]), need(s_w16[1])
            for c in (58, 62, 66, 70):
                mm16(c)
            # seg5: phase 3 fp16
            need(s16[3])
            for c in (59, 63, 67, 71):
                mm16(c)
            # seg6: fp8 DR pairs, phases (2,3) (closes the A chain).
            # After the 2nd DR23 issues, the fp16 A16 writebacks (hi psA
            # planes) are certainly drained - signal the early hi squares.
            need(s8[1]), need(s_w8[2])
            for kk, pair in enumerate(PAIRS23):
                inst = mmDR(pair)
                if kk == 1:
                    inst.then_inc(s_hi, 1)
            # segB: all of group B at the end; the A epilogue overlaps this
            for jb, c in enumerate(CHB):
                tensor.matmul(psBv[:], lhsT=wbt[:, jb * 64:(jb + 1) * 64],
                              rhs=rhs16(c), start=(jb == 0),
                              stop=(jb == len(CHB) - 1), skip_group_check=True)
            tensor.drain().then_inc(s_pe, 1)

    nc.compile()
    return nc


def pack_x(x):
    """x [B, 64000] f32 -> per-core (x16 [4,P,NI*QW] f16, x8 [2,P,2*NI*QW] e4m3)."""
    xp = np.pad(np.asarray(x, dtype=np.float32), ((0, 0), (PAD, PAD)))
    # column-major with phase-deinterleave: x_cm[b, p, r, q] = xp[b, (4q+r)*128+p]
    x_cm = xp.reshape(B, FW // 4, 4, P).transpose(0, 3, 2, 1)  # [B,128,4,157]
    packs = []
    for core in range(N_CORES):
        blk = x_cm[core * NI:(core + 1) * NI]            # [NI, 128, 4, 157]
        t = blk.transpose(2, 1, 0, 3)                    # [4, 128, NI, 157]
        p16 = np.ascontiguousarray(t.reshape(4, P, NI * QW)).astype(NP16)
        p8 = np.ascontiguousarray(t.reshape(2, 2, P, NI * QW)
                                  .transpose(0, 2, 1, 3)
                                  .reshape(2, P, 2 * NI * QW)).astype(NP8)
        packs.append((p16, p8))
    return packs


_PROGRAM = None


def _get_program():
    global _PROGRAM
    if _PROGRAM is None:
        _PROGRAM = build_program()
    return _PROGRAM


def run(x, **spmd_kwargs):
    """Run on 8 NeuronCores; returns (output [32, 84, 126] f32, results)."""
    nc = _get_program()
    packs = pack_x(x)
    in_maps = [{"x16_in": packs[i][0], "x8_in": packs[i][1],
                "wb_in": WB, "w16_in": W16, "w8_in": W8, "cn_in": CN}
               for i in range(N_CORES)]
    res = run_bass_kernel_spmd(nc, in_maps, core_ids=list(range(N_CORES)),
                               **spmd_kwargs)
    out = np.concatenate(
        [res.results[i]["out"].reshape(N_BINS, NI, T).transpose(1, 0, 2)
         for i in range(N_CORES)], axis=0)
    return np.ascontiguousarray(out.astype(np.float32)), res


def kernel(x):
    return run(x)[0]
